# revision 13
# baseline (speedup 1.0000x reference)
"""FEDFormer forward for nn_FEDFormer_7421703487916 on 8 trn2 NeuronCores.

Data-parallel over the fused (bs*channels)=256 batch axis, 32 per core.
The nine big (8224,512)@(512,512) projections (token-embed, and per layer:
q-proj, wo-proj, FF1, FF2 — ~85% of total FLOPs) run on-device through one
compiled Bass/Tile matmul kernel (fp32r single-pass PE matmuls, K-tiled
PSUM accumulation). Host numpy handles the batch-independent glue between
projections: rFFT/mode-mix/irFFT (length-257 prime FFT), series
decomposition moving-average, layernorm and the tiny decoder head.

The SPMD executable is jitted ONCE and reused for all nine dispatches
(run_bass_kernel_spmd rebuilds jax.jit(shard_map(...)) per call, which
re-traces and recompiles the NEFF wrapper every time — ~11.5 s/dispatch).
Output buffers are zero-filled on-device instead of shipping 134 MB of
host zeros per call.
"""

import numpy as np
from scipy.special import erf

import concourse.bass as bass
import concourse.mybir as mybir
import concourse.tile as tile
from concourse import bacc

# Problem constants (hardcoded per the harness contract).
B, T, CH, CIN = 16, 256, 16, 64
D, H, E, NL, M = 512, 8, 64, 2, 64
L = T + 1                     # 257
BE = B * CH                   # 256
N_CORES = 8
BSH = BE // N_CORES           # 32 batch rows per core
NT = BSH * L                  # 8224 tokens per core
K_MA = 25

_RUN = None                   # cached (fn, zero_fn, in_names, out_names)
_RUN_FF = None                # cached fused FF1->gelu->FF2 runner
_DEV_NS = 0.0                 # accumulated device-call wall time (ns)


def _build_nc():
    f32 = mybir.dt.float32
    f16 = mybir.dt.float16
    nc = bacc.Bacc("TRN2", target_bir_lowering=False, debug=False,
                   num_devices=N_CORES)
    at = nc.dram_tensor("at", (D, NT), f16, kind="ExternalInput").ap()
    bw = nc.dram_tensor("bw", (D, D), f16, kind="ExternalInput").ap()
    ct = nc.dram_tensor("ct", (D, NT), f16, kind="ExternalOutput").ap()

    KT = D // 128              # 4 contraction tiles
    OT = D // 128              # 4 output row tiles
    chunks = [(i * 512, min(512, NT - i * 512)) for i in range((NT + 511) // 512)]

    with tile.TileContext(nc) as tc:
        with (
            tc.tile_pool(name="aw", bufs=1) as apool,
            tc.tile_pool(name="bwp", bufs=1) as bpool,
            tc.tile_pool(name="out", bufs=4) as opool,
            tc.tile_pool(name="ps", bufs=8, space="PSUM") as pspool,
        ):
            a_sb = []
            b_sb = []
            for kt in range(KT):
                ta = apool.tile([128, NT], f16, tag=f"a{kt}")
                nc.sync.dma_start(ta[:], at[kt * 128:(kt + 1) * 128, :])
                a_sb.append(ta)
                tb = bpool.tile([128, D], f16, tag=f"b{kt}")
                nc.sync.dma_start(tb[:], bw[kt * 128:(kt + 1) * 128, :])
                b_sb.append(tb)
            for ot in range(OT):
                for (c0, w) in chunks:
                    ps = pspool.tile([128, 512], f32)
                    for kt in range(KT):
                        nc.tensor.matmul(
                            ps[:, :w],
                            b_sb[kt][:, ot * 128:(ot + 1) * 128],
                            a_sb[kt][:, c0:c0 + w],
                            start=(kt == 0), stop=(kt == KT - 1),
                        )
                    so = opool.tile([128, 512], f16)
                    nc.vector.tensor_copy(so[:, :w], ps[:, :w])
                    nc.sync.dma_start(ct[ot * 128:(ot + 1) * 128, c0:c0 + w],
                                      so[:, :w])
    nc.compile()
    return nc


def _build_nc_ff():
    """Fused FF: ct = gelu(at.T @ b1).T-chain -> @ b2, all on-device."""
    f32 = mybir.dt.float32
    f16 = mybir.dt.float16
    gelu_f = mybir.ActivationFunctionType.Gelu
    nc = bacc.Bacc("TRN2", target_bir_lowering=False, debug=False,
                   num_devices=N_CORES)
    at = nc.dram_tensor("at", (D, NT), f16, kind="ExternalInput").ap()
    b1 = nc.dram_tensor("b1", (D, D), f16, kind="ExternalInput").ap()
    b2 = nc.dram_tensor("b2", (D, D), f16, kind="ExternalInput").ap()
    ct = nc.dram_tensor("ct", (D, NT), f16, kind="ExternalOutput").ap()

    KT = D // 128
    OT = D // 128
    chunks = [(i * 512, min(512, NT - i * 512)) for i in range((NT + 511) // 512)]

    with tile.TileContext(nc) as tc:
        with (
            tc.tile_pool(name="aw", bufs=1) as apool,
            tc.tile_pool(name="bwp", bufs=1) as bpool,
            tc.tile_pool(name="gp", bufs=1) as gpool,
            tc.tile_pool(name="out", bufs=4) as opool,
            tc.tile_pool(name="ps", bufs=8, space="PSUM") as pspool,
        ):
            a_sb, b1_sb, b2_sb, g_sb = [], [], [], []
            for kt in range(KT):
                ta = apool.tile([128, NT], f16, tag=f"a{kt}")
                nc.sync.dma_start(ta[:], at[kt * 128:(kt + 1) * 128, :])
                a_sb.append(ta)
                t1 = bpool.tile([128, D], f16, tag=f"b1{kt}")
                nc.sync.dma_start(t1[:], b1[kt * 128:(kt + 1) * 128, :])
                b1_sb.append(t1)
                t2 = bpool.tile([128, D], f16, tag=f"b2{kt}")
                nc.sync.dma_start(t2[:], b2[kt * 128:(kt + 1) * 128, :])
                b2_sb.append(t2)
                tg = gpool.tile([128, NT], f16, tag=f"g{kt}")
                g_sb.append(tg)
            for ot in range(OT):
                for (c0, w) in chunks:
                    ps = pspool.tile([128, 512], f32)
                    for kt in range(KT):
                        nc.tensor.matmul(
                            ps[:, :w],
                            b1_sb[kt][:, ot * 128:(ot + 1) * 128],
                            a_sb[kt][:, c0:c0 + w],
                            start=(kt == 0), stop=(kt == KT - 1),
                        )
                    nc.scalar.activation(g_sb[ot][:, c0:c0 + w], ps[:, :w],
                                         func=gelu_f)
            for ot in range(OT):
                for (c0, w) in chunks:
                    ps = pspool.tile([128, 512], f32)
                    for kt in range(KT):
                        nc.tensor.matmul(
                            ps[:, :w],
                            b2_sb[kt][:, ot * 128:(ot + 1) * 128],
                            g_sb[kt][:, c0:c0 + w],
                            start=(kt == 0), stop=(kt == KT - 1),
                        )
                    so = opool.tile([128, 512], f16)
                    nc.vector.tensor_copy(so[:, :w], ps[:, :w])
                    nc.sync.dma_start(ct[ot * 128:(ot + 1) * 128, c0:c0 + w],
                                      so[:, :w])
    nc.compile()
    return nc


def _build_runner(nc_builder=_build_nc):
    """Compile the SPMD executable once; return a reusable dispatch fn."""
    import jax
    import jax.numpy as jnp
    from jax.experimental.shard_map import shard_map
    from jax.sharding import Mesh, NamedSharding, PartitionSpec
    from concourse.bass2jax import (_bass_exec_p, install_neuronx_cc_hook,
                                    partition_id_tensor)

    nc = nc_builder()
    install_neuronx_cc_hook()
    partition_name = (nc.partition_id_tensor.name
                      if nc.partition_id_tensor else None)
    in_names, in_specs_np, out_names, out_avals = [], [], [], []
    for alloc in nc.m.functions[0].allocations:
        if not isinstance(alloc, mybir.MemoryLocationSet):
            continue
        name = alloc.memorylocations[0].name
        if alloc.kind == "ExternalInput":
            if name != partition_name:
                in_names.append(name)
                in_specs_np.append((tuple(alloc.tensor_shape),
                                    mybir.dt.np(alloc.dtype)))
        elif alloc.kind == "ExternalOutput":
            out_names.append(name)
            out_avals.append(jax.core.ShapedArray(
                tuple(alloc.tensor_shape), mybir.dt.np(alloc.dtype)))
    n_params = len(in_names)
    n_outs = len(out_names)
    all_names = list(in_names) + list(out_names)
    if partition_name is not None:
        all_names.append(partition_name)
    donate = tuple(range(n_params, n_params + n_outs))

    def _body(*args):
        operands = list(args)
        if partition_name is not None:
            operands.append(partition_id_tensor())
        outs = _bass_exec_p.bind(
            *operands,
            out_avals=tuple(out_avals),
            in_names=tuple(all_names),
            out_names=tuple(out_names),
            lowering_input_output_aliases=(),
            sim_require_finite=True,
            sim_require_nnan=True,
            nc=nc,
        )
        return tuple(outs)

    devices = jax.devices()[:N_CORES]
    mesh = Mesh(np.asarray(devices), ("core",))
    in_specs = (PartitionSpec("core"),) * (n_params + n_outs)
    out_specs = (PartitionSpec("core"),) * n_outs
    fn = jax.jit(
        shard_map(_body, mesh=mesh, in_specs=in_specs,
                  out_specs=out_specs, check_rep=False),
        donate_argnums=donate, keep_unused=True)

    # On-device zero-fill for the donated output buffers (avoids shipping
    # 134 MB of host zeros per dispatch).
    shard = NamedSharding(mesh, PartitionSpec("core"))
    zero_fns = [
        jax.jit(lambda a=a: jnp.zeros((N_CORES * a.shape[0],) + a.shape[1:],
                                      a.dtype), out_shardings=shard)
        for a in out_avals
    ]

    # Warm up: one compile-triggering dispatch on zeros (setup, untimed —
    # the analogue of nc.compile() for the PJRT wrapper).
    zin = [np.zeros((N_CORES * shp[0],) + shp[1:], dt)
           for (shp, dt) in in_specs_np]
    outs = fn(*zin, *[zf() for zf in zero_fns])
    for o in outs:
        np.asarray(o)
    return fn, zero_fns, in_names, out_names


def _get_runner():
    global _RUN
    if _RUN is None:
        _RUN = _build_runner()
    return _RUN


def _mm(x, w):
    """x (N,512) @ w (512,512) on the 8 cores, rows sharded 8 ways."""
    global _DEV_NS
    import time
    fn, zero_fns, in_names, out_names = _get_runner()
    n = x.shape[0]
    sh = n // N_CORES
    wc = np.ascontiguousarray(w, dtype=np.float16)
    at = np.ascontiguousarray(
        np.asarray(x, np.float32).reshape(N_CORES, sh, D).transpose(0, 2, 1),
        dtype=np.float16,
    ).reshape(N_CORES * D, sh)
    bw = np.broadcast_to(wc, (N_CORES, D, D)).reshape(N_CORES * D, D)
    bw = np.ascontiguousarray(bw)
    ins = {"at": at, "bw": bw}
    args = [ins[name] for name in in_names]
    t0 = time.perf_counter()
    outs = fn(*args, *[zf() for zf in zero_fns])
    res = [np.asarray(o) for o in outs]
    _DEV_NS += (time.perf_counter() - t0) * 1e9
    ct = res[out_names.index("ct")].reshape(N_CORES, D, sh)
    return np.ascontiguousarray(
        ct.transpose(0, 2, 1).astype(np.float32)).reshape(n, D)


def _get_runner_ff():
    global _RUN_FF
    if _RUN_FF is None:
        _RUN_FF = _build_runner(_build_nc_ff)
    return _RUN_FF


def _ff(x, w1, w2):
    """gelu(x @ w1) @ w2 fused on-device, rows sharded 8 ways."""
    global _DEV_NS
    import time
    fn, zero_fns, in_names, out_names = _get_runner_ff()
    n = x.shape[0]
    sh = n // N_CORES
    at = np.ascontiguousarray(
        np.asarray(x, np.float32).reshape(N_CORES, sh, D).transpose(0, 2, 1),
        dtype=np.float16).reshape(N_CORES * D, sh)
    bws = {}
    for nm, w in (("b1", w1), ("b2", w2)):
        wc = np.ascontiguousarray(w, dtype=np.float16)
        bws[nm] = np.ascontiguousarray(
            np.broadcast_to(wc, (N_CORES, D, D)).reshape(N_CORES * D, D))
    ins = {"at": at, **bws}
    args = [ins[name] for name in in_names]
    t0 = time.perf_counter()
    outs = fn(*args, *[zf() for zf in zero_fns])
    res = [np.asarray(o) for o in outs]
    _DEV_NS += (time.perf_counter() - t0) * 1e9
    ct = res[out_names.index("ct")].reshape(N_CORES, D, sh)
    return np.ascontiguousarray(
        ct.transpose(0, 2, 1).astype(np.float32)).reshape(n, D)


def _pos_embed():
    pos = np.arange(L, dtype=np.float32)[:, None]
    div = np.exp(np.arange(0, D, 2, dtype=np.float32) * (-np.log(10000.0) / D))
    ang = pos * div
    pe = np.zeros((L, D), np.float32)
    pe[:, 0::2] = np.sin(ang)
    pe[:, 1::2] = np.cos(ang)
    return pe


def _moving_mean(v, k=K_MA):
    pad = (k - 1) // 2
    vp = np.concatenate([np.repeat(v[:, :1], pad, 1), v,
                         np.repeat(v[:, -1:], pad, 1)], axis=1)
    c = np.cumsum(vp, axis=1, dtype=np.float32)
    c = np.concatenate([np.zeros_like(c[:, :1]), c], axis=1)
    return (c[:, k:] - c[:, :-k]) / np.float32(k)


def _gelu(x):
    return (x * 0.5 * (1.0 + erf(x / np.sqrt(2.0, dtype=np.float32)))).astype(
        np.float32)


def kernel(x, p, y, cls, tok_w, wq, bq, wo, bo, conv1_w, conv2_w,
           four_wr, four_wi, norm_g, norm_b, dec1_w, dec1_b, dec2_w, dec2_b):
    x = np.asarray(x, np.float32)
    # cls prepend + channel fold: (BE, L, CIN)
    xc = np.concatenate(
        [np.broadcast_to(np.asarray(cls, np.float32), (B, CH, 1, CIN)),
         np.transpose(x, (0, 2, 1, 3))], axis=2).reshape(BE, L, CIN)
    # circular conv k=3 as one matmul: [roll+1 | x | roll-1] @ [w0;w1;w2]
    x3 = np.concatenate([np.roll(xc, 1, axis=1), xc,
                         np.roll(xc, -1, axis=1)], axis=2).reshape(BE * L, 3 * CIN)
    x3p = np.zeros((BE * L, D), np.float32)
    x3p[:, :3 * CIN] = x3
    tw = np.asarray(tok_w, np.float32)
    wtok = np.zeros((D, D), np.float32)
    wtok[:CIN, :] = tw[:, :, 0].T
    wtok[CIN:2 * CIN, :] = tw[:, :, 1].T
    wtok[2 * CIN:3 * CIN, :] = tw[:, :, 2].T
    h = _mm(x3p, wtok).reshape(BE, L, D) + _pos_embed()[None]

    w_cplx = np.asarray(four_wr, np.float32) + 1j * np.asarray(four_wi, np.float32)
    for l in range(NL):
        q = _mm(h.reshape(BE * L, D), np.asarray(wq[l], np.float32).T)
        q = q + np.asarray(bq[l], np.float32)
        xq = q.reshape(BE, L, H, E).transpose(0, 2, 3, 1)       # (BE,H,E,L)
        x_ft = np.fft.rfft(xq, axis=-1)
        sel = np.einsum('bhim,hiom->bhom', x_ft[..., :M], w_cplx)
        out_ft = np.zeros(x_ft.shape, np.complex128)
        out_ft[..., :M] = sel
        a = np.fft.irfft(out_ft, n=L, axis=-1).astype(np.float32)
        a = a.reshape(BE, L, H * E)                              # torch .view
        a2 = _mm(a.reshape(BE * L, D), np.asarray(wo[l], np.float32).T)
        a2 = a2 + np.asarray(bo[l], np.float32)
        h = h + a2.reshape(BE, L, D)
        h = h - _moving_mean(h)
        yff = _ff(h.reshape(BE * L, D),
                  np.asarray(conv1_w[l], np.float32).T,
                  np.asarray(conv2_w[l], np.float32).T)
        s2 = h + yff.reshape(BE, L, D)
        h = s2 - _moving_mean(s2)

    mu = np.mean(h, -1, keepdims=True)
    var = np.var(h, -1, keepdims=True)
    h = (h - mu) / np.sqrt(var + 1e-5) * np.asarray(norm_g, np.float32) \
        + np.asarray(norm_b, np.float32)
    z = np.mean(h, axis=1).reshape(B, CH * D)
    z = _gelu(z @ np.asarray(dec1_w, np.float32).T + np.asarray(dec1_b, np.float32))
    z = z @ np.asarray(dec2_w, np.float32).T + np.asarray(dec2_b, np.float32)
    return z[:, 0].astype(np.float32)



# revision 14
# speedup vs baseline: 1.2474x; 1.2474x over previous
"""FEDFormer forward for nn_FEDFormer_7421703487916 on 8 trn2 NeuronCores.

Data-parallel over the fused (bs*channels)=256 batch axis, 32 per core.
The nine big (8224,512)@(512,512) projections (token-embed, and per layer:
q-proj, wo-proj, FF1, FF2 — ~85% of total FLOPs) run on-device through one
compiled Bass/Tile matmul kernel (fp32r single-pass PE matmuls, K-tiled
PSUM accumulation). Host numpy handles the batch-independent glue between
projections: rFFT/mode-mix/irFFT (length-257 prime FFT), series
decomposition moving-average, layernorm and the tiny decoder head.

The SPMD executable is jitted ONCE and reused for all nine dispatches
(run_bass_kernel_spmd rebuilds jax.jit(shard_map(...)) per call, which
re-traces and recompiles the NEFF wrapper every time — ~11.5 s/dispatch).
Output buffers are zero-filled on-device instead of shipping 134 MB of
host zeros per call.
"""

import numpy as np
from scipy.special import erf

import concourse.bass as bass
import concourse.mybir as mybir
import concourse.tile as tile
from concourse import bacc

# Problem constants (hardcoded per the harness contract).
B, T, CH, CIN = 16, 256, 16, 64
D, H, E, NL, M = 512, 8, 64, 2, 64
L = T + 1                     # 257
BE = B * CH                   # 256
N_CORES = 8
BSH = BE // N_CORES           # 32 batch rows per core
NT = BSH * L                  # 8224 tokens per core
K_MA = 25

_RUN = None                   # cached (fn, zero_fn, in_names, out_names)
_RUN_FF = None                # cached fused FF1->gelu->FF2 runner
_DEV_NS = 0.0                 # accumulated device-call wall time (ns)


def _build_nc():
    f32 = mybir.dt.float32
    f16 = mybir.dt.float16
    nc = bacc.Bacc("TRN2", target_bir_lowering=False, debug=False,
                   num_devices=N_CORES)
    at = nc.dram_tensor("at", (D, NT), f16, kind="ExternalInput").ap()
    bw = nc.dram_tensor("bw", (D, D), f16, kind="ExternalInput").ap()
    ct = nc.dram_tensor("ct", (D, NT), f16, kind="ExternalOutput").ap()

    KT = D // 128              # 4 contraction tiles
    OT = D // 128              # 4 output row tiles
    chunks = [(i * 512, min(512, NT - i * 512)) for i in range((NT + 511) // 512)]

    with tile.TileContext(nc) as tc:
        with (
            tc.tile_pool(name="aw", bufs=1) as apool,
            tc.tile_pool(name="bwp", bufs=1) as bpool,
            tc.tile_pool(name="out", bufs=4) as opool,
            tc.tile_pool(name="ps", bufs=8, space="PSUM") as pspool,
        ):
            a_sb = []
            b_sb = []
            for kt in range(KT):
                ta = apool.tile([128, NT], f16, tag=f"a{kt}")
                nc.sync.dma_start(ta[:], at[kt * 128:(kt + 1) * 128, :])
                a_sb.append(ta)
                tb = bpool.tile([128, D], f16, tag=f"b{kt}")
                nc.sync.dma_start(tb[:], bw[kt * 128:(kt + 1) * 128, :])
                b_sb.append(tb)
            for ot in range(OT):
                for (c0, w) in chunks:
                    ps = pspool.tile([128, 512], f32)
                    for kt in range(KT):
                        nc.tensor.matmul(
                            ps[:, :w],
                            b_sb[kt][:, ot * 128:(ot + 1) * 128],
                            a_sb[kt][:, c0:c0 + w],
                            start=(kt == 0), stop=(kt == KT - 1),
                        )
                    so = opool.tile([128, 512], f16)
                    nc.vector.tensor_copy(so[:, :w], ps[:, :w])
                    nc.sync.dma_start(ct[ot * 128:(ot + 1) * 128, c0:c0 + w],
                                      so[:, :w])
    nc.compile()
    return nc


def _build_nc_ff():
    """Fused FF: ct = gelu(at.T @ b1).T-chain -> @ b2, all on-device."""
    f32 = mybir.dt.float32
    f16 = mybir.dt.float16
    gelu_f = mybir.ActivationFunctionType.Gelu
    nc = bacc.Bacc("TRN2", target_bir_lowering=False, debug=False,
                   num_devices=N_CORES)
    at = nc.dram_tensor("at", (D, NT), f16, kind="ExternalInput").ap()
    b1 = nc.dram_tensor("b1", (D, D), f16, kind="ExternalInput").ap()
    b2 = nc.dram_tensor("b2", (D, D), f16, kind="ExternalInput").ap()
    ct = nc.dram_tensor("ct", (D, NT), f16, kind="ExternalOutput").ap()

    KT = D // 128
    OT = D // 128
    chunks = [(i * 512, min(512, NT - i * 512)) for i in range((NT + 511) // 512)]

    with tile.TileContext(nc) as tc:
        with (
            tc.tile_pool(name="aw", bufs=1) as apool,
            tc.tile_pool(name="bwp", bufs=1) as bpool,
            tc.tile_pool(name="gp", bufs=1) as gpool,
            tc.tile_pool(name="out", bufs=4) as opool,
            tc.tile_pool(name="ps", bufs=8, space="PSUM") as pspool,
        ):
            a_sb, b1_sb, b2_sb, g_sb = [], [], [], []
            for kt in range(KT):
                ta = apool.tile([128, NT], f16, tag=f"a{kt}")
                nc.sync.dma_start(ta[:], at[kt * 128:(kt + 1) * 128, :])
                a_sb.append(ta)
                t1 = bpool.tile([128, D], f16, tag=f"b1{kt}")
                nc.sync.dma_start(t1[:], b1[kt * 128:(kt + 1) * 128, :])
                b1_sb.append(t1)
                t2 = bpool.tile([128, D], f16, tag=f"b2{kt}")
                nc.sync.dma_start(t2[:], b2[kt * 128:(kt + 1) * 128, :])
                b2_sb.append(t2)
                tg = gpool.tile([128, NT], f16, tag=f"g{kt}")
                g_sb.append(tg)
            for ot in range(OT):
                for (c0, w) in chunks:
                    ps = pspool.tile([128, 512], f32)
                    for kt in range(KT):
                        nc.tensor.matmul(
                            ps[:, :w],
                            b1_sb[kt][:, ot * 128:(ot + 1) * 128],
                            a_sb[kt][:, c0:c0 + w],
                            start=(kt == 0), stop=(kt == KT - 1),
                        )
                    nc.scalar.activation(g_sb[ot][:, c0:c0 + w], ps[:, :w],
                                         func=gelu_f)
            for ot in range(OT):
                for (c0, w) in chunks:
                    ps = pspool.tile([128, 512], f32)
                    for kt in range(KT):
                        nc.tensor.matmul(
                            ps[:, :w],
                            b2_sb[kt][:, ot * 128:(ot + 1) * 128],
                            g_sb[kt][:, c0:c0 + w],
                            start=(kt == 0), stop=(kt == KT - 1),
                        )
                    so = opool.tile([128, 512], f16)
                    nc.vector.tensor_copy(so[:, :w], ps[:, :w])
                    nc.sync.dma_start(ct[ot * 128:(ot + 1) * 128, c0:c0 + w],
                                      so[:, :w])
    nc.compile()
    return nc


def _build_runner(nc_builder=_build_nc):
    """Compile the SPMD executable once; return a reusable dispatch fn."""
    import jax
    import jax.numpy as jnp
    from jax.experimental.shard_map import shard_map
    from jax.sharding import Mesh, NamedSharding, PartitionSpec
    from concourse.bass2jax import (_bass_exec_p, install_neuronx_cc_hook,
                                    partition_id_tensor)

    nc = nc_builder()
    install_neuronx_cc_hook()
    partition_name = (nc.partition_id_tensor.name
                      if nc.partition_id_tensor else None)
    in_names, in_specs_np, out_names, out_avals = [], [], [], []
    for alloc in nc.m.functions[0].allocations:
        if not isinstance(alloc, mybir.MemoryLocationSet):
            continue
        name = alloc.memorylocations[0].name
        if alloc.kind == "ExternalInput":
            if name != partition_name:
                in_names.append(name)
                in_specs_np.append((tuple(alloc.tensor_shape),
                                    mybir.dt.np(alloc.dtype)))
        elif alloc.kind == "ExternalOutput":
            out_names.append(name)
            out_avals.append(jax.core.ShapedArray(
                tuple(alloc.tensor_shape), mybir.dt.np(alloc.dtype)))
    n_params = len(in_names)
    n_outs = len(out_names)
    all_names = list(in_names) + list(out_names)
    if partition_name is not None:
        all_names.append(partition_name)
    donate = tuple(range(n_params, n_params + n_outs))

    def _body(*args):
        operands = list(args)
        if partition_name is not None:
            operands.append(partition_id_tensor())
        outs = _bass_exec_p.bind(
            *operands,
            out_avals=tuple(out_avals),
            in_names=tuple(all_names),
            out_names=tuple(out_names),
            lowering_input_output_aliases=(),
            sim_require_finite=True,
            sim_require_nnan=True,
            nc=nc,
        )
        return tuple(outs)

    devices = jax.devices()[:N_CORES]
    mesh = Mesh(np.asarray(devices), ("core",))
    in_specs = (PartitionSpec("core"),) * (n_params + n_outs)
    out_specs = (PartitionSpec("core"),) * n_outs
    fn = jax.jit(
        shard_map(_body, mesh=mesh, in_specs=in_specs,
                  out_specs=out_specs, check_rep=False),
        donate_argnums=donate, keep_unused=True)

    # On-device zero-fill for the donated output buffers (avoids shipping
    # 134 MB of host zeros per dispatch).
    shard = NamedSharding(mesh, PartitionSpec("core"))
    zero_fns = [
        jax.jit(lambda a=a: jnp.zeros((N_CORES * a.shape[0],) + a.shape[1:],
                                      a.dtype), out_shardings=shard)
        for a in out_avals
    ]

    # Warm up: one compile-triggering dispatch on zeros (setup, untimed —
    # the analogue of nc.compile() for the PJRT wrapper).
    zin = [np.zeros((N_CORES * shp[0],) + shp[1:], dt)
           for (shp, dt) in in_specs_np]
    outs = fn(*zin, *[zf() for zf in zero_fns])
    for o in outs:
        np.asarray(o)
    return fn, zero_fns, in_names, out_names


def _get_runner():
    global _RUN
    if _RUN is None:
        _RUN = _build_runner()
    return _RUN


def _mm(x, w):
    """x (N,512) @ w (512,512) on the 8 cores, rows sharded 8 ways."""
    global _DEV_NS
    import time
    fn, zero_fns, in_names, out_names = _get_runner()
    n = x.shape[0]
    sh = n // N_CORES
    wc = np.ascontiguousarray(w, dtype=np.float16)
    at = np.ascontiguousarray(
        np.asarray(x, np.float32).reshape(N_CORES, sh, D).transpose(0, 2, 1),
        dtype=np.float16,
    ).reshape(N_CORES * D, sh)
    bw = np.broadcast_to(wc, (N_CORES, D, D)).reshape(N_CORES * D, D)
    bw = np.ascontiguousarray(bw)
    ins = {"at": at, "bw": bw}
    args = [ins[name] for name in in_names]
    t0 = time.perf_counter()
    outs = fn(*args, *[zf() for zf in zero_fns])
    res = [np.asarray(o) for o in outs]
    _DEV_NS += (time.perf_counter() - t0) * 1e9
    ct = res[out_names.index("ct")].reshape(N_CORES, D, sh)
    return np.ascontiguousarray(
        ct.transpose(0, 2, 1).astype(np.float32)).reshape(n, D)


def _get_runner_ff():
    global _RUN_FF
    if _RUN_FF is None:
        _RUN_FF = _build_runner(_build_nc_ff)
    return _RUN_FF


def _ff(x, w1, w2):
    """gelu(x @ w1) @ w2 fused on-device, rows sharded 8 ways."""
    global _DEV_NS
    import time
    fn, zero_fns, in_names, out_names = _get_runner_ff()
    n = x.shape[0]
    sh = n // N_CORES
    at = np.ascontiguousarray(
        np.asarray(x, np.float32).reshape(N_CORES, sh, D).transpose(0, 2, 1),
        dtype=np.float16).reshape(N_CORES * D, sh)
    bws = {}
    for nm, w in (("b1", w1), ("b2", w2)):
        wc = np.ascontiguousarray(w, dtype=np.float16)
        bws[nm] = np.ascontiguousarray(
            np.broadcast_to(wc, (N_CORES, D, D)).reshape(N_CORES * D, D))
    ins = {"at": at, **bws}
    args = [ins[name] for name in in_names]
    t0 = time.perf_counter()
    outs = fn(*args, *[zf() for zf in zero_fns])
    res = [np.asarray(o) for o in outs]
    _DEV_NS += (time.perf_counter() - t0) * 1e9
    ct = res[out_names.index("ct")].reshape(N_CORES, D, sh)
    return np.ascontiguousarray(
        ct.transpose(0, 2, 1).astype(np.float32)).reshape(n, D)


def _pos_embed():
    pos = np.arange(L, dtype=np.float32)[:, None]
    div = np.exp(np.arange(0, D, 2, dtype=np.float32) * (-np.log(10000.0) / D))
    ang = pos * div
    pe = np.zeros((L, D), np.float32)
    pe[:, 0::2] = np.sin(ang)
    pe[:, 1::2] = np.cos(ang)
    return pe


def _moving_mean(v, k=K_MA):
    pad = (k - 1) // 2
    vp = np.concatenate([np.repeat(v[:, :1], pad, 1), v,
                         np.repeat(v[:, -1:], pad, 1)], axis=1)
    c = np.cumsum(vp, axis=1, dtype=np.float32)
    c = np.concatenate([np.zeros_like(c[:, :1]), c], axis=1)
    return (c[:, k:] - c[:, :-k]) / np.float32(k)


def _gelu(x):
    return (x * 0.5 * (1.0 + erf(x / np.sqrt(2.0, dtype=np.float32)))).astype(
        np.float32)


def kernel(x, p, y, cls, tok_w, wq, bq, wo, bo, conv1_w, conv2_w,
           four_wr, four_wi, norm_g, norm_b, dec1_w, dec1_b, dec2_w, dec2_b):
    x = np.asarray(x, np.float32)
    # cls prepend + channel fold: (BE, L, CIN)
    xc = np.concatenate(
        [np.broadcast_to(np.asarray(cls, np.float32), (B, CH, 1, CIN)),
         np.transpose(x, (0, 2, 1, 3))], axis=2).reshape(BE, L, CIN)
    # circular conv k=3 as one matmul: [roll+1 | x | roll-1] @ [w0;w1;w2]
    x3 = np.concatenate([np.roll(xc, 1, axis=1), xc,
                         np.roll(xc, -1, axis=1)], axis=2).reshape(BE * L, 3 * CIN)
    x3p = np.zeros((BE * L, D), np.float32)
    x3p[:, :3 * CIN] = x3
    tw = np.asarray(tok_w, np.float32)
    wtok = np.zeros((D, D), np.float32)
    wtok[:CIN, :] = tw[:, :, 0].T
    wtok[CIN:2 * CIN, :] = tw[:, :, 1].T
    wtok[2 * CIN:3 * CIN, :] = tw[:, :, 2].T
    h = _mm(x3p, wtok).reshape(BE, L, D) + _pos_embed()[None]

    w_cplx = np.asarray(four_wr, np.float32) + 1j * np.asarray(four_wi, np.float32)
    for l in range(NL):
        q = _mm(h.reshape(BE * L, D), np.asarray(wq[l], np.float32).T)
        q = q + np.asarray(bq[l], np.float32)
        xq = q.reshape(BE, L, H, E).transpose(0, 2, 3, 1)       # (BE,H,E,L)
        x_ft = np.fft.rfft(xq, axis=-1)
        sel = np.einsum('bhim,hiom->bhom', x_ft[..., :M], w_cplx)
        out_ft = np.zeros(x_ft.shape, np.complex128)
        out_ft[..., :M] = sel
        a = np.fft.irfft(out_ft, n=L, axis=-1).astype(np.float32)
        a = a.reshape(BE, L, H * E)                              # torch .view
        a2 = _mm(a.reshape(BE * L, D), np.asarray(wo[l], np.float32).T)
        a2 = a2 + np.asarray(bo[l], np.float32)
        h = h + a2.reshape(BE, L, D)
        h = h - _moving_mean(h)
        f1 = _mm(h.reshape(BE * L, D), np.asarray(conv1_w[l], np.float32).T)
        yff = _mm(_gelu(f1), np.asarray(conv2_w[l], np.float32).T)
        s2 = h + yff.reshape(BE, L, D)
        h = s2 - _moving_mean(s2)

    mu = np.mean(h, -1, keepdims=True)
    var = np.var(h, -1, keepdims=True)
    h = (h - mu) / np.sqrt(var + 1e-5) * np.asarray(norm_g, np.float32) \
        + np.asarray(norm_b, np.float32)
    z = np.mean(h, axis=1).reshape(B, CH * D)
    z = _gelu(z @ np.asarray(dec1_w, np.float32).T + np.asarray(dec1_b, np.float32))
    z = z @ np.asarray(dec2_w, np.float32).T + np.asarray(dec2_b, np.float32)
    return z[:, 0].astype(np.float32)

